# revision 21
# baseline (speedup 1.0000x reference)
"""AttentionHyperNet kernel — data-parallel across 8 NeuronCores.

Wire-optimized path: the tunnel to the device pod is the bottleneck
(~80 MB/s up, ~50 MB/s down, ~70 ms per RPC round trip), so the kernel
  * drops masked entity rows on the host (they cannot affect the
    output: masked agents are zeroed, masked entities get -inf
    attention logits) and ships only the valid rows, flat-packed, as
    float16 — the device re-expands them to a padded per-sample layout
    with a gather,
  * folds Wout@W2 into one (128,32) matrix on the host (no
    nonlinearity between them) and moves the output bias to the host,
  * packs everything into two sharded float16 buffers (payload +
    metadata) so packing overlaps the upload and no replicated
    transfers happen,
  * computes in f32 on device (exp-masked softmax without the
    max-subtraction passes; logit scale folded into the q weights),
    quantizes the compacted valid output rows to int8 with a per-core
    scale (error <= 1/254 of the per-core max, well under the 2e-2
    global-max tolerance),
  * scatters the dequantized rows into the full fp32 (4096, 64, 32)
    output on the host.

Self-contained: no sibling imports, shapes hardcoded.
"""

import os
import sys
import time

import numpy as np

N_AGENTS = 64
N_HEADS = 4
N_CORES = 8
BS = 4096
NE = 128
FD = 19
E = 128
M = 32
SH = BS // N_CORES
HD = E // N_HEADS

# metadata layout (f16 elements), all sections 128-aligned
N_PAR = FD * E + E + E * 3 * E + E * M
N_PAR_PAD = ((N_PAR + 127) // 128) * 128
O_CNT = N_PAR_PAD  # then: cnt (sh), off_hi (sh), off_lo (sh), idx hi/lo
N_CHUNKS = 4

_DEBUG = bool(os.environ.get("BASSKERNEL_DEBUG"))


def _dbg(msg):
    if _DEBUG:
        print(f"[kernel] {msg}", file=sys.stderr, flush=True)


def _round_up(x, m):
    return ((int(x) + m - 1) // m) * m


_JAX_STATE = {}
_FWD_CACHE = {}


def _jax_state():
    if _JAX_STATE:
        return _JAX_STATE
    import jax
    from jax.sharding import Mesh, NamedSharding, PartitionSpec as P

    devs = jax.devices()[:N_CORES]
    if len(devs) < N_CORES:
        raise RuntimeError("need 8 cores")
    mesh = Mesh(np.array(devs), ("b",))
    _JAX_STATE["jax"] = jax
    _JAX_STATE["mesh"] = mesh
    _JAX_STATE["shard"] = NamedSharding(mesh, P("b"))
    _JAX_STATE["P"] = P
    return _JAX_STATE


def _get_fwd(sh, cap_e, cap_a, ne_pad, na_pad):
    key = (sh, cap_e, cap_a, ne_pad, na_pad)
    fn = _FWD_CACHE.get(key)
    if fn is not None:
        return fn
    st = _jax_state()
    jax = st["jax"]
    mesh = st["mesh"]
    P = st["P"]
    import jax.numpy as jnp

    from jax.experimental.shard_map import shard_map

    n_ent = cap_e * FD

    def core_fwd(pay, meta):  # (1, n_ent) f16, (1, meta_total) f16
        pay = pay.reshape(-1)
        meta = meta.reshape(-1)
        ent = pay[:n_ent].reshape(cap_e, FD)
        pos = [0]

        def take(n, shape):
            v = meta[pos[0] : pos[0] + n].reshape(shape)
            pos[0] += n
            return v

        W1 = take(FD * E, (FD, E)).astype(jnp.float32)
        b1 = take(E, (E,)).astype(jnp.float32)
        Wqkv = take(E * 3 * E, (E, 3 * E)).astype(jnp.float32)
        Wc = take(E * M, (E, M)).astype(jnp.float32)
        pos[0] = O_CNT
        cnt = take(sh, (sh,)).astype(jnp.float32)
        off = (
            take(sh, (sh,)).astype(jnp.float32) * 256.0
            + take(sh, (sh,)).astype(jnp.float32)
        ).astype(jnp.int32)
        oidx = (
            take(cap_a, (cap_a,)).astype(jnp.float32) * 256.0
            + take(cap_a, (cap_a,)).astype(jnp.float32)
        ).astype(jnp.int32)

        gidx = jnp.clip(
            off[:, None] + jnp.arange(ne_pad, dtype=jnp.int32)[None, :],
            0,
            cap_e - 1,
        )
        pe = ent.astype(jnp.float32)[gidx]  # (sh, ne_pad, FD) f32
        x1 = jax.nn.relu(pe @ W1 + b1)
        qkv = x1 @ Wqkv  # (sh, ne_pad, 3E)
        q = qkv[:, :na_pad, :E]
        k = qkv[:, :, E : 2 * E]
        v = qkv[:, :, 2 * E :]
        qh = q.reshape(sh, na_pad, N_HEADS, HD)
        kh = k.reshape(sh, ne_pad, N_HEADS, HD)
        vh = v.reshape(sh, ne_pad, N_HEADS, HD)
        logits = jnp.einsum("sqhd,skhd->shqk", qh, kh)
        kmask = (
            jnp.arange(ne_pad, dtype=jnp.float32)[None, :] < cnt[:, None]
        ).astype(jnp.float32)
        ex = jnp.exp(logits) * kmask[:, None, None, :]
        w = ex / (jnp.sum(ex, axis=-1, keepdims=True) + 1e-30)
        attn = jnp.einsum("shqk,skhd->sqhd", w, vh).reshape(sh, na_pad, E)
        x3 = jnp.einsum("sqe,em->sqm", attn, Wc)
        flat = x3.reshape(sh * na_pad, M)
        out = flat[oidx]  # (cap_a, M) f32, pad slots duplicate row 0
        smax = jnp.max(jnp.abs(out))
        scale = jnp.maximum(smax, 1e-20) * (1.0 / 127.0)
        qv = jnp.clip(jnp.rint(out / scale), -127, 127).astype(jnp.int8)
        return qv[None], scale.reshape(1, 1)

    fwd = jax.jit(
        shard_map(
            core_fwd,
            mesh=mesh,
            in_specs=(P("b"), P("b")),
            out_specs=(P("b"), P("b")),
            check_rep=False,
        )
    )
    _FWD_CACHE[key] = fwd
    return fwd


def _run_packed(entities, entity_mask, W1, b1, Wqkv, Wout, bout, W2, b2):
    st = _jax_state()
    jax = st["jax"]
    shard = st["shard"]

    t0 = time.perf_counter()
    ent = np.ascontiguousarray(entities, np.float32).reshape(BS, NE, FD)
    valid = np.ascontiguousarray(entity_mask).reshape(BS, NE) == 0
    cnt_e = valid.sum(1).astype(np.int64)
    va = valid[:, :N_AGENTS]
    cnt_a = va.sum(1).astype(np.int64)
    shc = SH // N_CHUNKS  # samples per core per chunk
    ce = cnt_e.reshape(N_CORES, N_CHUNKS, shc)
    ca = cnt_a.reshape(N_CORES, N_CHUNKS, shc)
    seg_tot_e = ce.sum(2)  # (C, T)
    seg_tot_a = ca.sum(2)

    ne_pad = max(8, _round_up(ce.max(), 8))
    na_pad = max(8, _round_up(ca.max(), 8))
    cap_e = max(2048, _round_up(seg_tot_e.max(), 2048))
    cap_a = max(1024, _round_up(seg_tot_a.max(), 1024))
    n_ent = cap_e * FD
    o_off_hi = O_CNT + shc
    o_off_lo = o_off_hi + shc
    o_idx = o_off_lo + shc
    meta_total = _round_up(o_idx + 2 * cap_a + 256, 128)

    from concurrent.futures import ThreadPoolExecutor

    devs = st["mesh"].devices.reshape(-1)
    if "pool" not in st:
        st["pool"] = ThreadPoolExecutor(max_workers=N_CORES)
    pool = st["pool"]

    # host-folded params (shared by all chunks)
    Wc = (
        np.asarray(Wout, np.float64) @ np.asarray(W2, np.float64)
    ).astype(np.float32)
    bc = (
        np.asarray(bout, np.float64) @ np.asarray(W2, np.float64)
        + np.asarray(b2, np.float64)
    ).astype(np.float32)
    Wqkv_s = np.asarray(Wqkv, np.float32).copy()
    Wqkv_s[:, :E] *= 1.0 / np.sqrt(float(HD))  # fold logit scale into q
    params16 = np.concatenate(
        [
            np.asarray(W1, np.float32).ravel(),
            np.asarray(b1, np.float32).ravel(),
            Wqkv_s.ravel(),
            Wc.ravel(),
        ]
    ).astype(np.float16)

    fwd = _get_fwd(shc, cap_e, cap_a, ne_pad, na_pad)

    def pack_pay_one(c, t):
        arr = np.zeros((1, n_ent), np.float16)
        g0 = c * SH + t * shc
        rows = ent[g0 : g0 + shc][valid[g0 : g0 + shc]]
        n = rows.shape[0]
        if n:
            arr[0, : n * FD] = rows.astype(np.float16).reshape(-1)
        return jax.device_put(arr, devs[c])

    chunk_outs = []
    marks = [t0]
    for t in range(N_CHUNKS):
        futs = [pool.submit(pack_pay_one, c, t) for c in range(N_CORES)]
        meta = np.zeros((N_CORES, meta_total), np.float16)
        meta[:, :N_PAR] = params16[None]
        meta[:, O_CNT : O_CNT + shc] = ce[:, t].astype(np.float16)
        for c in range(N_CORES):
            seg_cum = np.zeros(shc + 1, np.int64)
            np.cumsum(ce[c, t], out=seg_cum[1:])
            off = seg_cum[:-1]
            meta[c, o_off_hi : o_off_hi + shc] = (off >> 8).astype(np.float16)
            meta[c, o_off_lo : o_off_lo + shc] = (off & 255).astype(np.float16)
            ta = int(seg_tot_a[c, t])
            if ta:
                i_ids = np.repeat(np.arange(shc), ca[c, t])
                cum_a = np.zeros(shc + 1, np.int64)
                np.cumsum(ca[c, t], out=cum_a[1:])
                j_ids = np.arange(ta) - np.repeat(cum_a[:-1], ca[c, t])
                idx = i_ids * na_pad + j_ids
                meta[c, o_idx : o_idx + ta] = (idx >> 8).astype(np.float16)
                meta[c, o_idx + cap_a : o_idx + cap_a + ta] = (
                    idx & 255
                ).astype(np.float16)
        g_meta = jax.device_put(meta, shard)
        singles = [f.result() for f in futs]
        g_pay = jax.make_array_from_single_device_arrays(
            (N_CORES, n_ent), shard, singles
        )
        outq, outs = fwd(g_pay, g_meta)
        chunk_outs.append((outq, outs))
        marks.append(time.perf_counter())

    # issue all host copies, then drain chunk by chunk
    all_datas = []
    for outq, outs in chunk_outs:
        qs = sorted(
            outq.addressable_shards, key=lambda s: s.index[0].start or 0
        )
        ss = sorted(
            outs.addressable_shards, key=lambda s: s.index[0].start or 0
        )
        ds = [s.data for s in qs] + [s.data for s in ss]
        for d in ds:
            d.copy_to_host_async()
        all_datas.append(ds)
    marks.append(time.perf_counter())

    res = np.zeros((BS, N_AGENTS, M), np.float32)
    sample_chunk = (np.arange(BS) % SH) // shc  # chunk id per sample
    for t in range(N_CHUNKS):
        vals = [np.asarray(d) for d in all_datas[t]]
        tot_a = int(seg_tot_a[:, t].sum())
        picked = np.empty((tot_a, M), np.float32)
        bnd = np.zeros(N_CORES + 1, np.int64)
        np.cumsum(seg_tot_a[:, t], out=bnd[1:])

        def dequant_one(c):
            ta = int(seg_tot_a[c, t])
            seg = picked[bnd[c] : bnd[c] + ta]
            np.multiply(
                vals[c].reshape(cap_a, M)[:ta].astype(np.float32),
                float(vals[N_CORES + c].reshape(-1)[0]),
                out=seg,
            )
            seg += bc[None, :]

        list(pool.map(dequant_one, range(N_CORES)))
        res[va & (sample_chunk == t)[:, None]] = picked
        marks.append(time.perf_counter())
    _dbg(
        "chunks "
        + " ".join(f"{b - a:.3f}" for a, b in zip(marks, marks[1:]))
        + f" total:{marks[-1] - marks[0]:.3f}"
    )
    return res


def _forward_np(entities, entity_mask, W1, b1, Wqkv, Wout, bout, W2, b2):
    bs, ne, _ = entities.shape
    x1 = np.maximum(entities @ W1 + b1, 0.0)
    em = entity_mask.astype(np.float32)
    am = em[:, :N_AGENTS]
    attn_mask = 1.0 - np.einsum("bi,bj->bij", 1.0 - am, 1.0 - em)
    qkv = x1 @ Wqkv
    q, k, v = np.split(qkv, 3, axis=-1)
    q = q[:, :N_AGENTS]

    def heads(t):
        b, n, _ = t.shape
        return t.reshape(b, n, N_HEADS, HD).transpose(0, 2, 1, 3)

    qh, kh, vh = heads(q), heads(k), heads(v)
    logits = np.einsum("bhqd,bhkd->bhqk", qh, kh) / np.sqrt(np.float32(HD))
    logits = np.where(attn_mask[:, None] > 0, -np.inf, logits)
    m = np.max(logits, axis=-1, keepdims=True)
    m = np.where(np.isinf(m), 0.0, m)
    ex = np.exp(logits - m)
    s = np.sum(ex, axis=-1, keepdims=True)
    w = np.where(s > 0, ex / np.where(s == 0, 1.0, s), 0.0)
    attn = np.einsum("bhqk,bhkd->bhqd", w, vh)
    attn = attn.transpose(0, 2, 1, 3).reshape(bs, N_AGENTS, E)
    x2 = attn @ Wout + bout
    x2 = np.where(am[:, :, None] > 0, 0.0, x2)
    x3 = x2 @ W2 + b2
    x3 = np.where(am[:, :, None] > 0, 0.0, x3)
    return x3.astype(np.float32)


def kernel(entities, entity_mask, W1, b1, Wqkv, Wout, bout, W2, b2):
    args = (
        np.asarray(entities, np.float32),
        np.asarray(entity_mask, np.int32),
        np.asarray(W1, np.float32),
        np.asarray(b1, np.float32),
        np.asarray(Wqkv, np.float32),
        np.asarray(Wout, np.float32),
        np.asarray(bout, np.float32),
        np.asarray(W2, np.float32),
        np.asarray(b2, np.float32),
    )
    try:
        return _run_packed(*args)
    except Exception as e:
        _dbg(f"packed path failed: {type(e).__name__}: {e}")
        return _forward_np(*args)


# revision 22
# speedup vs baseline: 1.0997x; 1.0997x over previous
"""AttentionHyperNet kernel — data-parallel across 8 NeuronCores.

Wire-optimized path: the tunnel to the device pod is the bottleneck
(~80 MB/s up, ~50 MB/s down, ~70 ms per RPC round trip), so the kernel
  * drops masked entity rows on the host (they cannot affect the
    output: masked agents are zeroed, masked entities get -inf
    attention logits) and ships only the valid rows, flat-packed, as
    float16 — the device re-expands them to a padded per-sample layout
    with a gather,
  * folds Wout@W2 into one (128,32) matrix on the host (no
    nonlinearity between them) and moves the output bias to the host,
  * packs everything into two sharded float16 buffers (payload +
    metadata) so packing overlaps the upload and no replicated
    transfers happen,
  * computes in f32 on device (exp-masked softmax without the
    max-subtraction passes; logit scale folded into the q weights),
    quantizes the compacted valid output rows to int8 with a per-core
    scale (error <= 1/254 of the per-core max, well under the 2e-2
    global-max tolerance),
  * scatters the dequantized rows into the full fp32 (4096, 64, 32)
    output on the host.

Self-contained: no sibling imports, shapes hardcoded.
"""

import os
import sys
import time

import numpy as np

N_AGENTS = 64
N_HEADS = 4
N_CORES = 8
BS = 4096
NE = 128
FD = 19
E = 128
M = 32
SH = BS // N_CORES
HD = E // N_HEADS

# metadata layout (f16 elements), all sections 128-aligned
N_PAR = FD * E + E + E * 3 * E + E * M
N_PAR_PAD = ((N_PAR + 127) // 128) * 128
O_CNT = N_PAR_PAD  # then: cnt (sh), off_hi (sh), off_lo (sh), idx hi/lo
N_CHUNKS = 2

_DEBUG = bool(os.environ.get("BASSKERNEL_DEBUG"))


def _dbg(msg):
    if _DEBUG:
        print(f"[kernel] {msg}", file=sys.stderr, flush=True)


def _round_up(x, m):
    return ((int(x) + m - 1) // m) * m


_JAX_STATE = {}
_FWD_CACHE = {}


def _jax_state():
    if _JAX_STATE:
        return _JAX_STATE
    import jax
    from jax.sharding import Mesh, NamedSharding, PartitionSpec as P

    devs = jax.devices()[:N_CORES]
    if len(devs) < N_CORES:
        raise RuntimeError("need 8 cores")
    mesh = Mesh(np.array(devs), ("b",))
    _JAX_STATE["jax"] = jax
    _JAX_STATE["mesh"] = mesh
    _JAX_STATE["shard"] = NamedSharding(mesh, P("b"))
    _JAX_STATE["P"] = P
    return _JAX_STATE


def _get_fwd(sh, cap_e, cap_a, ne_pad, na_pad):
    key = (sh, cap_e, cap_a, ne_pad, na_pad)
    fn = _FWD_CACHE.get(key)
    if fn is not None:
        return fn
    st = _jax_state()
    jax = st["jax"]
    mesh = st["mesh"]
    P = st["P"]
    import jax.numpy as jnp

    from jax.experimental.shard_map import shard_map

    n_ent = cap_e * FD

    def core_fwd(pay, meta):  # (1, n_ent) f16, (1, meta_total) f16
        pay = pay.reshape(-1)
        meta = meta.reshape(-1)
        ent = pay[:n_ent].reshape(cap_e, FD)
        pos = [0]

        def take(n, shape):
            v = meta[pos[0] : pos[0] + n].reshape(shape)
            pos[0] += n
            return v

        W1 = take(FD * E, (FD, E)).astype(jnp.float32)
        b1 = take(E, (E,)).astype(jnp.float32)
        Wqkv = take(E * 3 * E, (E, 3 * E)).astype(jnp.float32)
        Wc = take(E * M, (E, M)).astype(jnp.float32)
        pos[0] = O_CNT
        cnt = take(sh, (sh,)).astype(jnp.float32)
        off = (
            take(sh, (sh,)).astype(jnp.float32) * 256.0
            + take(sh, (sh,)).astype(jnp.float32)
        ).astype(jnp.int32)
        oidx = (
            take(cap_a, (cap_a,)).astype(jnp.float32) * 256.0
            + take(cap_a, (cap_a,)).astype(jnp.float32)
        ).astype(jnp.int32)

        gidx = jnp.clip(
            off[:, None] + jnp.arange(ne_pad, dtype=jnp.int32)[None, :],
            0,
            cap_e - 1,
        )
        pe = ent.astype(jnp.float32)[gidx]  # (sh, ne_pad, FD) f32
        x1 = jax.nn.relu(pe @ W1 + b1)
        qkv = x1 @ Wqkv  # (sh, ne_pad, 3E)
        q = qkv[:, :na_pad, :E]
        k = qkv[:, :, E : 2 * E]
        v = qkv[:, :, 2 * E :]
        qh = q.reshape(sh, na_pad, N_HEADS, HD)
        kh = k.reshape(sh, ne_pad, N_HEADS, HD)
        vh = v.reshape(sh, ne_pad, N_HEADS, HD)
        logits = jnp.einsum("sqhd,skhd->shqk", qh, kh)
        kmask = (
            jnp.arange(ne_pad, dtype=jnp.float32)[None, :] < cnt[:, None]
        ).astype(jnp.float32)
        ex = jnp.exp(logits) * kmask[:, None, None, :]
        w = ex / (jnp.sum(ex, axis=-1, keepdims=True) + 1e-30)
        attn = jnp.einsum("shqk,skhd->sqhd", w, vh).reshape(sh, na_pad, E)
        x3 = jnp.einsum("sqe,em->sqm", attn, Wc)
        flat = x3.reshape(sh * na_pad, M)
        out = flat[oidx]  # (cap_a, M) f32, pad slots duplicate row 0
        smax = jnp.max(jnp.abs(out))
        scale = jnp.maximum(smax, 1e-20) * (1.0 / 127.0)
        qv = jnp.clip(jnp.rint(out / scale), -127, 127).astype(jnp.int8)
        return qv[None], scale.reshape(1, 1)

    fwd = jax.jit(
        shard_map(
            core_fwd,
            mesh=mesh,
            in_specs=(P("b"), P("b")),
            out_specs=(P("b"), P("b")),
            check_rep=False,
        )
    )
    _FWD_CACHE[key] = fwd
    return fwd


def _run_packed(entities, entity_mask, W1, b1, Wqkv, Wout, bout, W2, b2):
    st = _jax_state()
    jax = st["jax"]
    shard = st["shard"]

    t0 = time.perf_counter()
    ent = np.ascontiguousarray(entities, np.float32).reshape(BS, NE, FD)
    valid = np.ascontiguousarray(entity_mask).reshape(BS, NE) == 0
    cnt_e = valid.sum(1).astype(np.int64)
    va = valid[:, :N_AGENTS]
    cnt_a = va.sum(1).astype(np.int64)
    shc = SH // N_CHUNKS  # samples per core per chunk
    ce = cnt_e.reshape(N_CORES, N_CHUNKS, shc)
    ca = cnt_a.reshape(N_CORES, N_CHUNKS, shc)
    seg_tot_e = ce.sum(2)  # (C, T)
    seg_tot_a = ca.sum(2)

    ne_pad = max(8, _round_up(ce.max(), 8))
    na_pad = max(8, _round_up(ca.max(), 8))
    cap_e = max(2048, _round_up(seg_tot_e.max(), 2048))
    cap_a = max(1024, _round_up(seg_tot_a.max(), 1024))
    n_ent = cap_e * FD
    o_off_hi = O_CNT + shc
    o_off_lo = o_off_hi + shc
    o_idx = o_off_lo + shc
    meta_total = _round_up(o_idx + 2 * cap_a + 256, 128)

    from concurrent.futures import ThreadPoolExecutor

    devs = st["mesh"].devices.reshape(-1)
    if "pool" not in st:
        st["pool"] = ThreadPoolExecutor(max_workers=N_CORES)
    pool = st["pool"]

    # host-folded params (shared by all chunks)
    Wc = (
        np.asarray(Wout, np.float64) @ np.asarray(W2, np.float64)
    ).astype(np.float32)
    bc = (
        np.asarray(bout, np.float64) @ np.asarray(W2, np.float64)
        + np.asarray(b2, np.float64)
    ).astype(np.float32)
    Wqkv_s = np.asarray(Wqkv, np.float32).copy()
    Wqkv_s[:, :E] *= 1.0 / np.sqrt(float(HD))  # fold logit scale into q
    params16 = np.concatenate(
        [
            np.asarray(W1, np.float32).ravel(),
            np.asarray(b1, np.float32).ravel(),
            Wqkv_s.ravel(),
            Wc.ravel(),
        ]
    ).astype(np.float16)

    fwd = _get_fwd(shc, cap_e, cap_a, ne_pad, na_pad)

    def pack_pay_one(c, t):
        arr = np.zeros((1, n_ent), np.float16)
        g0 = c * SH + t * shc
        rows = ent[g0 : g0 + shc][valid[g0 : g0 + shc]]
        n = rows.shape[0]
        if n:
            arr[0, : n * FD] = rows.astype(np.float16).reshape(-1)
        return jax.device_put(arr, devs[c])

    chunk_outs = []
    marks = [t0]
    for t in range(N_CHUNKS):
        futs = [pool.submit(pack_pay_one, c, t) for c in range(N_CORES)]
        meta = np.zeros((N_CORES, meta_total), np.float16)
        meta[:, :N_PAR] = params16[None]
        meta[:, O_CNT : O_CNT + shc] = ce[:, t].astype(np.float16)
        for c in range(N_CORES):
            seg_cum = np.zeros(shc + 1, np.int64)
            np.cumsum(ce[c, t], out=seg_cum[1:])
            off = seg_cum[:-1]
            meta[c, o_off_hi : o_off_hi + shc] = (off >> 8).astype(np.float16)
            meta[c, o_off_lo : o_off_lo + shc] = (off & 255).astype(np.float16)
            ta = int(seg_tot_a[c, t])
            if ta:
                i_ids = np.repeat(np.arange(shc), ca[c, t])
                cum_a = np.zeros(shc + 1, np.int64)
                np.cumsum(ca[c, t], out=cum_a[1:])
                j_ids = np.arange(ta) - np.repeat(cum_a[:-1], ca[c, t])
                idx = i_ids * na_pad + j_ids
                meta[c, o_idx : o_idx + ta] = (idx >> 8).astype(np.float16)
                meta[c, o_idx + cap_a : o_idx + cap_a + ta] = (
                    idx & 255
                ).astype(np.float16)
        g_meta = jax.device_put(meta, shard)
        singles = [f.result() for f in futs]
        g_pay = jax.make_array_from_single_device_arrays(
            (N_CORES, n_ent), shard, singles
        )
        outq, outs = fwd(g_pay, g_meta)
        chunk_outs.append((outq, outs))
        marks.append(time.perf_counter())

    # issue all host copies, then drain chunk by chunk
    all_datas = []
    for outq, outs in chunk_outs:
        qs = sorted(
            outq.addressable_shards, key=lambda s: s.index[0].start or 0
        )
        ss = sorted(
            outs.addressable_shards, key=lambda s: s.index[0].start or 0
        )
        ds = [s.data for s in qs] + [s.data for s in ss]
        for d in ds:
            d.copy_to_host_async()
        all_datas.append(ds)
    marks.append(time.perf_counter())

    res = np.zeros((BS, N_AGENTS, M), np.float32)
    sample_chunk = (np.arange(BS) % SH) // shc  # chunk id per sample
    for t in range(N_CHUNKS):
        vals = [np.asarray(d) for d in all_datas[t]]
        tot_a = int(seg_tot_a[:, t].sum())
        picked = np.empty((tot_a, M), np.float32)
        bnd = np.zeros(N_CORES + 1, np.int64)
        np.cumsum(seg_tot_a[:, t], out=bnd[1:])

        def dequant_one(c):
            ta = int(seg_tot_a[c, t])
            seg = picked[bnd[c] : bnd[c] + ta]
            np.multiply(
                vals[c].reshape(cap_a, M)[:ta].astype(np.float32),
                float(vals[N_CORES + c].reshape(-1)[0]),
                out=seg,
            )
            seg += bc[None, :]

        list(pool.map(dequant_one, range(N_CORES)))
        res[va & (sample_chunk == t)[:, None]] = picked
        marks.append(time.perf_counter())
    _dbg(
        "chunks "
        + " ".join(f"{b - a:.3f}" for a, b in zip(marks, marks[1:]))
        + f" total:{marks[-1] - marks[0]:.3f}"
    )
    return res


def _forward_np(entities, entity_mask, W1, b1, Wqkv, Wout, bout, W2, b2):
    bs, ne, _ = entities.shape
    x1 = np.maximum(entities @ W1 + b1, 0.0)
    em = entity_mask.astype(np.float32)
    am = em[:, :N_AGENTS]
    attn_mask = 1.0 - np.einsum("bi,bj->bij", 1.0 - am, 1.0 - em)
    qkv = x1 @ Wqkv
    q, k, v = np.split(qkv, 3, axis=-1)
    q = q[:, :N_AGENTS]

    def heads(t):
        b, n, _ = t.shape
        return t.reshape(b, n, N_HEADS, HD).transpose(0, 2, 1, 3)

    qh, kh, vh = heads(q), heads(k), heads(v)
    logits = np.einsum("bhqd,bhkd->bhqk", qh, kh) / np.sqrt(np.float32(HD))
    logits = np.where(attn_mask[:, None] > 0, -np.inf, logits)
    m = np.max(logits, axis=-1, keepdims=True)
    m = np.where(np.isinf(m), 0.0, m)
    ex = np.exp(logits - m)
    s = np.sum(ex, axis=-1, keepdims=True)
    w = np.where(s > 0, ex / np.where(s == 0, 1.0, s), 0.0)
    attn = np.einsum("bhqk,bhkd->bhqd", w, vh)
    attn = attn.transpose(0, 2, 1, 3).reshape(bs, N_AGENTS, E)
    x2 = attn @ Wout + bout
    x2 = np.where(am[:, :, None] > 0, 0.0, x2)
    x3 = x2 @ W2 + b2
    x3 = np.where(am[:, :, None] > 0, 0.0, x3)
    return x3.astype(np.float32)


def kernel(entities, entity_mask, W1, b1, Wqkv, Wout, bout, W2, b2):
    args = (
        np.asarray(entities, np.float32),
        np.asarray(entity_mask, np.int32),
        np.asarray(W1, np.float32),
        np.asarray(b1, np.float32),
        np.asarray(Wqkv, np.float32),
        np.asarray(Wout, np.float32),
        np.asarray(bout, np.float32),
        np.asarray(W2, np.float32),
        np.asarray(b2, np.float32),
    )
    try:
        return _run_packed(*args)
    except Exception as e:
        _dbg(f"packed path failed: {type(e).__name__}: {e}")
        return _forward_np(*args)


# revision 23
# speedup vs baseline: 1.1204x; 1.0188x over previous
"""AttentionHyperNet kernel — data-parallel across 8 NeuronCores.

Wire-optimized path: the tunnel to the device pod is the bottleneck
(~80 MB/s up, ~50 MB/s down, ~70 ms per RPC round trip), so the kernel
  * drops masked entity rows on the host (they cannot affect the
    output: masked agents are zeroed, masked entities get -inf
    attention logits) and ships only the valid rows, flat-packed, as
    float16 — the device re-expands them to a padded per-sample layout
    with a gather,
  * folds Wout@W2 into one (128,32) matrix on the host (no
    nonlinearity between them) and moves the output bias to the host,
  * packs everything into two sharded float16 buffers (payload +
    metadata) so packing overlaps the upload and no replicated
    transfers happen,
  * computes in f32 on device (exp-masked softmax without the
    max-subtraction passes; logit scale folded into the q weights),
    quantizes the compacted valid output rows to int8 with a per-core
    scale (error <= 1/254 of the per-core max, well under the 2e-2
    global-max tolerance),
  * scatters the dequantized rows into the full fp32 (4096, 64, 32)
    output on the host.

Self-contained: no sibling imports, shapes hardcoded.
"""

import os
import sys
import time

import numpy as np

N_AGENTS = 64
N_HEADS = 4
N_CORES = 8
BS = 4096
NE = 128
FD = 19
E = 128
M = 32
SH = BS // N_CORES
HD = E // N_HEADS

# metadata layout (f16 elements), all sections 128-aligned
N_PAR = FD * E + E + E * 3 * E + E * M
N_PAR_PAD = ((N_PAR + 127) // 128) * 128
O_CNT = N_PAR_PAD  # then: cnt (sh), off_hi (sh), off_lo (sh), idx hi/lo
N_CHUNKS = 2

_DEBUG = bool(os.environ.get("BASSKERNEL_DEBUG"))


def _dbg(msg):
    if _DEBUG:
        print(f"[kernel] {msg}", file=sys.stderr, flush=True)


def _round_up(x, m):
    return ((int(x) + m - 1) // m) * m


_JAX_STATE = {}
_FWD_CACHE = {}


def _jax_state():
    if _JAX_STATE:
        return _JAX_STATE
    import jax
    from jax.sharding import Mesh, NamedSharding, PartitionSpec as P

    devs = jax.devices()[:N_CORES]
    if len(devs) < N_CORES:
        raise RuntimeError("need 8 cores")
    mesh = Mesh(np.array(devs), ("b",))
    _JAX_STATE["jax"] = jax
    _JAX_STATE["mesh"] = mesh
    _JAX_STATE["shard"] = NamedSharding(mesh, P("b"))
    _JAX_STATE["P"] = P
    return _JAX_STATE


def _get_fwd(sh, cap_e, cap_a, ne_pad, na_pad):
    key = (sh, cap_e, cap_a, ne_pad, na_pad)
    fn = _FWD_CACHE.get(key)
    if fn is not None:
        return fn
    st = _jax_state()
    jax = st["jax"]
    mesh = st["mesh"]
    P = st["P"]
    import jax.numpy as jnp

    from jax.experimental.shard_map import shard_map

    n_ent = cap_e * FD

    def core_fwd(buf):  # (1, n_ent + meta) f16, payload then metadata
        meta = buf.reshape(-1)
        ent = meta[:n_ent].reshape(cap_e, FD)
        pos = [n_ent]

        def take(n, shape):
            v = meta[pos[0] : pos[0] + n].reshape(shape)
            pos[0] += n
            return v

        W1 = take(FD * E, (FD, E)).astype(jnp.float32)
        b1 = take(E, (E,)).astype(jnp.float32)
        Wqkv = take(E * 3 * E, (E, 3 * E)).astype(jnp.float32)
        Wc = take(E * M, (E, M)).astype(jnp.float32)
        pos[0] = n_ent + O_CNT
        cnt = take(sh, (sh,)).astype(jnp.float32)
        off = (
            take(sh, (sh,)).astype(jnp.float32) * 256.0
            + take(sh, (sh,)).astype(jnp.float32)
        ).astype(jnp.int32)
        oidx = (
            take(cap_a, (cap_a,)).astype(jnp.float32) * 256.0
            + take(cap_a, (cap_a,)).astype(jnp.float32)
        ).astype(jnp.int32)

        gidx = jnp.clip(
            off[:, None] + jnp.arange(ne_pad, dtype=jnp.int32)[None, :],
            0,
            cap_e - 1,
        )
        pe = ent.astype(jnp.float32)[gidx]  # (sh, ne_pad, FD) f32
        x1 = jax.nn.relu(pe @ W1 + b1)
        qkv = x1 @ Wqkv  # (sh, ne_pad, 3E)
        q = qkv[:, :na_pad, :E]
        k = qkv[:, :, E : 2 * E]
        v = qkv[:, :, 2 * E :]
        qh = q.reshape(sh, na_pad, N_HEADS, HD)
        kh = k.reshape(sh, ne_pad, N_HEADS, HD)
        vh = v.reshape(sh, ne_pad, N_HEADS, HD)
        logits = jnp.einsum("sqhd,skhd->shqk", qh, kh)
        kmask = (
            jnp.arange(ne_pad, dtype=jnp.float32)[None, :] < cnt[:, None]
        ).astype(jnp.float32)
        ex = jnp.exp(logits) * kmask[:, None, None, :]
        w = ex / (jnp.sum(ex, axis=-1, keepdims=True) + 1e-30)
        attn = jnp.einsum("shqk,skhd->sqhd", w, vh).reshape(sh, na_pad, E)
        x3 = jnp.einsum("sqe,em->sqm", attn, Wc)
        flat = x3.reshape(sh * na_pad, M)
        out = flat[oidx]  # (cap_a, M) f32, pad slots duplicate row 0
        smax = jnp.max(jnp.abs(out))
        scale = jnp.maximum(smax, 1e-20) * (1.0 / 127.0)
        qv = jnp.clip(jnp.rint(out / scale), -127, 127).astype(jnp.int8)
        return qv[None], scale.reshape(1, 1)

    fwd = jax.jit(
        shard_map(
            core_fwd,
            mesh=mesh,
            in_specs=P("b"),
            out_specs=(P("b"), P("b")),
            check_rep=False,
        )
    )
    _FWD_CACHE[key] = fwd
    return fwd


def _run_packed(entities, entity_mask, W1, b1, Wqkv, Wout, bout, W2, b2):
    st = _jax_state()
    jax = st["jax"]
    shard = st["shard"]

    t0 = time.perf_counter()
    ent = np.ascontiguousarray(entities, np.float32).reshape(BS, NE, FD)
    valid = np.ascontiguousarray(entity_mask).reshape(BS, NE) == 0
    cnt_e = valid.sum(1).astype(np.int64)
    va = valid[:, :N_AGENTS]
    cnt_a = va.sum(1).astype(np.int64)
    shc = SH // N_CHUNKS  # samples per core per chunk
    ce = cnt_e.reshape(N_CORES, N_CHUNKS, shc)
    ca = cnt_a.reshape(N_CORES, N_CHUNKS, shc)
    seg_tot_e = ce.sum(2)  # (C, T)
    seg_tot_a = ca.sum(2)

    ne_pad = max(8, _round_up(ce.max(), 8))
    na_pad = max(8, _round_up(ca.max(), 8))
    cap_e = max(2048, _round_up(seg_tot_e.max(), 2048))
    cap_a = max(1024, _round_up(seg_tot_a.max(), 1024))
    n_ent = cap_e * FD
    o_off_hi = O_CNT + shc
    o_off_lo = o_off_hi + shc
    o_idx = o_off_lo + shc
    meta_total = _round_up(o_idx + 2 * cap_a + 256, 128)

    from concurrent.futures import ThreadPoolExecutor

    devs = st["mesh"].devices.reshape(-1)
    if "pool" not in st:
        st["pool"] = ThreadPoolExecutor(max_workers=N_CORES)
    pool = st["pool"]

    # host-folded params (shared by all chunks)
    Wc = (
        np.asarray(Wout, np.float64) @ np.asarray(W2, np.float64)
    ).astype(np.float32)
    bc = (
        np.asarray(bout, np.float64) @ np.asarray(W2, np.float64)
        + np.asarray(b2, np.float64)
    ).astype(np.float32)
    Wqkv_s = np.asarray(Wqkv, np.float32).copy()
    Wqkv_s[:, :E] *= 1.0 / np.sqrt(float(HD))  # fold logit scale into q
    params16 = np.concatenate(
        [
            np.asarray(W1, np.float32).ravel(),
            np.asarray(b1, np.float32).ravel(),
            Wqkv_s.ravel(),
            Wc.ravel(),
        ]
    ).astype(np.float16)

    fwd = _get_fwd(shc, cap_e, cap_a, ne_pad, na_pad)

    total = n_ent + meta_total

    def pack_one(c, t):
        arr = np.zeros((1, total), np.float16)
        g0 = c * SH + t * shc
        rows = ent[g0 : g0 + shc][valid[g0 : g0 + shc]]
        n = rows.shape[0]
        if n:
            arr[0, : n * FD] = rows.astype(np.float16).reshape(-1)
        mt = arr[0, n_ent:]
        mt[:N_PAR] = params16
        mt[O_CNT : O_CNT + shc] = ce[c, t].astype(np.float16)
        seg_cum = np.zeros(shc + 1, np.int64)
        np.cumsum(ce[c, t], out=seg_cum[1:])
        off = seg_cum[:-1]
        mt[o_off_hi : o_off_hi + shc] = (off >> 8).astype(np.float16)
        mt[o_off_lo : o_off_lo + shc] = (off & 255).astype(np.float16)
        ta = int(seg_tot_a[c, t])
        if ta:
            i_ids = np.repeat(np.arange(shc), ca[c, t])
            cum_a = np.zeros(shc + 1, np.int64)
            np.cumsum(ca[c, t], out=cum_a[1:])
            j_ids = np.arange(ta) - np.repeat(cum_a[:-1], ca[c, t])
            idx = i_ids * na_pad + j_ids
            mt[o_idx : o_idx + ta] = (idx >> 8).astype(np.float16)
            mt[o_idx + cap_a : o_idx + cap_a + ta] = (idx & 255).astype(
                np.float16
            )
        return jax.device_put(arr, devs[c])

    chunk_outs = []
    marks = [t0]
    for t in range(N_CHUNKS):
        futs = [pool.submit(pack_one, c, t) for c in range(N_CORES)]
        singles = [f.result() for f in futs]
        g_buf = jax.make_array_from_single_device_arrays(
            (N_CORES, total), shard, singles
        )
        outq, outs = fwd(g_buf)
        chunk_outs.append((outq, outs))
        marks.append(time.perf_counter())

    # issue all host copies, then drain chunk by chunk
    all_datas = []
    for outq, outs in chunk_outs:
        qs = sorted(
            outq.addressable_shards, key=lambda s: s.index[0].start or 0
        )
        ss = sorted(
            outs.addressable_shards, key=lambda s: s.index[0].start or 0
        )
        ds = [s.data for s in qs] + [s.data for s in ss]
        for d in ds:
            d.copy_to_host_async()
        all_datas.append(ds)
    marks.append(time.perf_counter())

    res = np.zeros((BS, N_AGENTS, M), np.float32)
    sample_chunk = (np.arange(BS) % SH) // shc  # chunk id per sample
    for t in range(N_CHUNKS):
        vals = [np.asarray(d) for d in all_datas[t]]
        tot_a = int(seg_tot_a[:, t].sum())
        picked = np.empty((tot_a, M), np.float32)
        bnd = np.zeros(N_CORES + 1, np.int64)
        np.cumsum(seg_tot_a[:, t], out=bnd[1:])

        def dequant_one(c):
            ta = int(seg_tot_a[c, t])
            seg = picked[bnd[c] : bnd[c] + ta]
            np.multiply(
                vals[c].reshape(cap_a, M)[:ta].astype(np.float32),
                float(vals[N_CORES + c].reshape(-1)[0]),
                out=seg,
            )
            seg += bc[None, :]

        list(pool.map(dequant_one, range(N_CORES)))
        res[va & (sample_chunk == t)[:, None]] = picked
        marks.append(time.perf_counter())
    _dbg(
        "chunks "
        + " ".join(f"{b - a:.3f}" for a, b in zip(marks, marks[1:]))
        + f" total:{marks[-1] - marks[0]:.3f}"
    )
    return res


def _forward_np(entities, entity_mask, W1, b1, Wqkv, Wout, bout, W2, b2):
    bs, ne, _ = entities.shape
    x1 = np.maximum(entities @ W1 + b1, 0.0)
    em = entity_mask.astype(np.float32)
    am = em[:, :N_AGENTS]
    attn_mask = 1.0 - np.einsum("bi,bj->bij", 1.0 - am, 1.0 - em)
    qkv = x1 @ Wqkv
    q, k, v = np.split(qkv, 3, axis=-1)
    q = q[:, :N_AGENTS]

    def heads(t):
        b, n, _ = t.shape
        return t.reshape(b, n, N_HEADS, HD).transpose(0, 2, 1, 3)

    qh, kh, vh = heads(q), heads(k), heads(v)
    logits = np.einsum("bhqd,bhkd->bhqk", qh, kh) / np.sqrt(np.float32(HD))
    logits = np.where(attn_mask[:, None] > 0, -np.inf, logits)
    m = np.max(logits, axis=-1, keepdims=True)
    m = np.where(np.isinf(m), 0.0, m)
    ex = np.exp(logits - m)
    s = np.sum(ex, axis=-1, keepdims=True)
    w = np.where(s > 0, ex / np.where(s == 0, 1.0, s), 0.0)
    attn = np.einsum("bhqk,bhkd->bhqd", w, vh)
    attn = attn.transpose(0, 2, 1, 3).reshape(bs, N_AGENTS, E)
    x2 = attn @ Wout + bout
    x2 = np.where(am[:, :, None] > 0, 0.0, x2)
    x3 = x2 @ W2 + b2
    x3 = np.where(am[:, :, None] > 0, 0.0, x3)
    return x3.astype(np.float32)


def kernel(entities, entity_mask, W1, b1, Wqkv, Wout, bout, W2, b2):
    args = (
        np.asarray(entities, np.float32),
        np.asarray(entity_mask, np.int32),
        np.asarray(W1, np.float32),
        np.asarray(b1, np.float32),
        np.asarray(Wqkv, np.float32),
        np.asarray(Wout, np.float32),
        np.asarray(bout, np.float32),
        np.asarray(W2, np.float32),
        np.asarray(b2, np.float32),
    )
    try:
        return _run_packed(*args)
    except Exception as e:
        _dbg(f"packed path failed: {type(e).__name__}: {e}")
        return _forward_np(*args)


# revision 24
# speedup vs baseline: 1.1665x; 1.0411x over previous
"""AttentionHyperNet kernel — data-parallel across 8 NeuronCores.

Wire-optimized path: the tunnel to the device pod is the bottleneck
(~80 MB/s up, ~50 MB/s down, ~70 ms per RPC round trip), so the kernel
  * drops masked entity rows on the host (they cannot affect the
    output: masked agents are zeroed, masked entities get -inf
    attention logits) and ships only the valid rows, flat-packed, as
    float16 — the device re-expands them to a padded per-sample layout
    with a gather,
  * folds Wout@W2 into one (128,32) matrix on the host (no
    nonlinearity between them) and moves the output bias to the host,
  * packs payload + params + offsets/indices into ONE float16 buffer
    per core (no replicated transfers, one put per core, built and
    uploaded from a thread pool), and runs the batch as two pipelined
    chunks per core so chunk B's upload overlaps chunk A's compute
    and chunk A's download,
  * computes in f32 on device (exp-masked softmax without the
    max-subtraction passes; logit scale folded into the q weights),
    quantizes the compacted valid output rows to int8 with a per-core
    scale (error <= 1/254 of the per-core max, well under the 2e-2
    global-max tolerance),
  * scatters the dequantized rows into the full fp32 (4096, 64, 32)
    output on the host.

Self-contained: no sibling imports, shapes hardcoded.
"""

import os
import sys
import time

import numpy as np

N_AGENTS = 64
N_HEADS = 4
N_CORES = 8
BS = 4096
NE = 128
FD = 19
E = 128
M = 32
SH = BS // N_CORES
HD = E // N_HEADS

# metadata layout (f16 elements), all sections 128-aligned
N_PAR = FD * E + E + E * 3 * E + E * M
N_PAR_PAD = ((N_PAR + 127) // 128) * 128
O_CNT = N_PAR_PAD  # then: cnt (sh), off_hi (sh), off_lo (sh), idx hi/lo
N_CHUNKS = 2

_DEBUG = bool(os.environ.get("BASSKERNEL_DEBUG"))


def _dbg(msg):
    if _DEBUG:
        print(f"[kernel] {msg}", file=sys.stderr, flush=True)


def _round_up(x, m):
    return ((int(x) + m - 1) // m) * m


_JAX_STATE = {}
_FWD_CACHE = {}


def _jax_state():
    if _JAX_STATE:
        return _JAX_STATE
    import jax
    from jax.sharding import Mesh, NamedSharding, PartitionSpec as P

    devs = jax.devices()[:N_CORES]
    if len(devs) < N_CORES:
        raise RuntimeError("need 8 cores")
    mesh = Mesh(np.array(devs), ("b",))
    _JAX_STATE["jax"] = jax
    _JAX_STATE["mesh"] = mesh
    _JAX_STATE["shard"] = NamedSharding(mesh, P("b"))
    _JAX_STATE["P"] = P
    return _JAX_STATE


def _get_fwd(sh, cap_e, cap_a, ne_pad, na_pad):
    key = (sh, cap_e, cap_a, ne_pad, na_pad)
    fn = _FWD_CACHE.get(key)
    if fn is not None:
        return fn
    st = _jax_state()
    jax = st["jax"]
    mesh = st["mesh"]
    P = st["P"]
    import jax.numpy as jnp

    from jax.experimental.shard_map import shard_map

    n_ent = cap_e * FD

    def core_fwd(buf):  # (1, n_ent + meta) f16, payload then metadata
        meta = buf.reshape(-1)
        ent = meta[:n_ent].reshape(cap_e, FD)
        pos = [n_ent]

        def take(n, shape):
            v = meta[pos[0] : pos[0] + n].reshape(shape)
            pos[0] += n
            return v

        W1 = take(FD * E, (FD, E)).astype(jnp.float32)
        b1 = take(E, (E,)).astype(jnp.float32)
        Wqkv = take(E * 3 * E, (E, 3 * E)).astype(jnp.float32)
        Wc = take(E * M, (E, M)).astype(jnp.float32)
        pos[0] = n_ent + O_CNT
        cnt = take(sh, (sh,)).astype(jnp.float32)
        off = (
            take(sh, (sh,)).astype(jnp.float32) * 256.0
            + take(sh, (sh,)).astype(jnp.float32)
        ).astype(jnp.int32)
        oidx = (
            take(cap_a, (cap_a,)).astype(jnp.float32) * 256.0
            + take(cap_a, (cap_a,)).astype(jnp.float32)
        ).astype(jnp.int32)

        gidx = jnp.clip(
            off[:, None] + jnp.arange(ne_pad, dtype=jnp.int32)[None, :],
            0,
            cap_e - 1,
        )
        pe = ent.astype(jnp.float32)[gidx]  # (sh, ne_pad, FD) f32
        x1 = jax.nn.relu(pe @ W1 + b1)
        qkv = x1 @ Wqkv  # (sh, ne_pad, 3E)
        q = qkv[:, :na_pad, :E]
        k = qkv[:, :, E : 2 * E]
        v = qkv[:, :, 2 * E :]
        qh = q.reshape(sh, na_pad, N_HEADS, HD)
        kh = k.reshape(sh, ne_pad, N_HEADS, HD)
        vh = v.reshape(sh, ne_pad, N_HEADS, HD)
        logits = jnp.einsum("sqhd,skhd->shqk", qh, kh)
        kmask = (
            jnp.arange(ne_pad, dtype=jnp.float32)[None, :] < cnt[:, None]
        ).astype(jnp.float32)
        ex = jnp.exp(logits) * kmask[:, None, None, :]
        w = ex / (jnp.sum(ex, axis=-1, keepdims=True) + 1e-30)
        attn = jnp.einsum("shqk,skhd->sqhd", w, vh).reshape(sh, na_pad, E)
        x3 = jnp.einsum("sqe,em->sqm", attn, Wc)
        flat = x3.reshape(sh * na_pad, M)
        out = flat[oidx]  # (cap_a, M) f32, pad slots duplicate row 0
        smax = jnp.max(jnp.abs(out))
        scale = jnp.maximum(smax, 1e-20) * (1.0 / 127.0)
        qv = jnp.clip(jnp.rint(out / scale), -127, 127).astype(jnp.int8)
        return qv[None], scale.reshape(1, 1)

    fwd = jax.jit(
        shard_map(
            core_fwd,
            mesh=mesh,
            in_specs=P("b"),
            out_specs=(P("b"), P("b")),
            check_rep=False,
        )
    )
    _FWD_CACHE[key] = fwd
    return fwd


def _run_packed(entities, entity_mask, W1, b1, Wqkv, Wout, bout, W2, b2):
    st = _jax_state()
    jax = st["jax"]
    shard = st["shard"]

    t0 = time.perf_counter()
    ent = np.ascontiguousarray(entities, np.float32).reshape(BS, NE, FD)
    valid = np.ascontiguousarray(entity_mask).reshape(BS, NE) == 0
    cnt_e = valid.sum(1).astype(np.int64)
    va = valid[:, :N_AGENTS]
    cnt_a = va.sum(1).astype(np.int64)
    shc = SH // N_CHUNKS  # samples per core per chunk
    ce = cnt_e.reshape(N_CORES, N_CHUNKS, shc)
    ca = cnt_a.reshape(N_CORES, N_CHUNKS, shc)
    seg_tot_e = ce.sum(2)  # (C, T)
    seg_tot_a = ca.sum(2)

    ne_pad = max(8, _round_up(ce.max(), 8))
    na_pad = max(8, _round_up(ca.max(), 8))
    cap_e = max(2048, _round_up(seg_tot_e.max(), 2048))
    cap_a = max(1024, _round_up(seg_tot_a.max(), 1024))
    n_ent = cap_e * FD
    o_off_hi = O_CNT + shc
    o_off_lo = o_off_hi + shc
    o_idx = o_off_lo + shc
    meta_total = _round_up(o_idx + 2 * cap_a + 256, 128)

    from concurrent.futures import ThreadPoolExecutor

    devs = st["mesh"].devices.reshape(-1)
    if "pool" not in st:
        st["pool"] = ThreadPoolExecutor(max_workers=N_CORES)
    pool = st["pool"]

    # host-folded params (shared by all chunks)
    Wc = (
        np.asarray(Wout, np.float64) @ np.asarray(W2, np.float64)
    ).astype(np.float32)
    bc = (
        np.asarray(bout, np.float64) @ np.asarray(W2, np.float64)
        + np.asarray(b2, np.float64)
    ).astype(np.float32)
    Wqkv_s = np.asarray(Wqkv, np.float32).copy()
    Wqkv_s[:, :E] *= 1.0 / np.sqrt(float(HD))  # fold logit scale into q
    params16 = np.concatenate(
        [
            np.asarray(W1, np.float32).ravel(),
            np.asarray(b1, np.float32).ravel(),
            Wqkv_s.ravel(),
            Wc.ravel(),
        ]
    ).astype(np.float16)

    fwd = _get_fwd(shc, cap_e, cap_a, ne_pad, na_pad)

    total = n_ent + meta_total

    def pack_one(c, t):
        arr = np.zeros((1, total), np.float16)
        g0 = c * SH + t * shc
        rows = ent[g0 : g0 + shc][valid[g0 : g0 + shc]]
        n = rows.shape[0]
        if n:
            arr[0, : n * FD] = rows.astype(np.float16).reshape(-1)
        mt = arr[0, n_ent:]
        mt[:N_PAR] = params16
        mt[O_CNT : O_CNT + shc] = ce[c, t].astype(np.float16)
        seg_cum = np.zeros(shc + 1, np.int64)
        np.cumsum(ce[c, t], out=seg_cum[1:])
        off = seg_cum[:-1]
        mt[o_off_hi : o_off_hi + shc] = (off >> 8).astype(np.float16)
        mt[o_off_lo : o_off_lo + shc] = (off & 255).astype(np.float16)
        ta = int(seg_tot_a[c, t])
        if ta:
            i_ids = np.repeat(np.arange(shc), ca[c, t])
            cum_a = np.zeros(shc + 1, np.int64)
            np.cumsum(ca[c, t], out=cum_a[1:])
            j_ids = np.arange(ta) - np.repeat(cum_a[:-1], ca[c, t])
            idx = i_ids * na_pad + j_ids
            mt[o_idx : o_idx + ta] = (idx >> 8).astype(np.float16)
            mt[o_idx + cap_a : o_idx + cap_a + ta] = (idx & 255).astype(
                np.float16
            )
        return jax.device_put(arr, devs[c])

    chunk_outs = []
    marks = [t0]
    for t in range(N_CHUNKS):
        futs = [pool.submit(pack_one, c, t) for c in range(N_CORES)]
        singles = [f.result() for f in futs]
        g_buf = jax.make_array_from_single_device_arrays(
            (N_CORES, total), shard, singles
        )
        outq, outs = fwd(g_buf)
        chunk_outs.append((outq, outs))
        marks.append(time.perf_counter())

    # issue all host copies, then drain chunk by chunk
    all_datas = []
    for outq, outs in chunk_outs:
        qs = sorted(
            outq.addressable_shards, key=lambda s: s.index[0].start or 0
        )
        ss = sorted(
            outs.addressable_shards, key=lambda s: s.index[0].start or 0
        )
        ds = [s.data for s in qs] + [s.data for s in ss]
        for d in ds:
            d.copy_to_host_async()
        all_datas.append(ds)
    marks.append(time.perf_counter())

    res = np.zeros((BS, N_AGENTS, M), np.float32)
    sample_chunk = (np.arange(BS) % SH) // shc  # chunk id per sample
    for t in range(N_CHUNKS):
        vals = [np.asarray(d) for d in all_datas[t]]
        tot_a = int(seg_tot_a[:, t].sum())
        picked = np.empty((tot_a, M), np.float32)
        bnd = np.zeros(N_CORES + 1, np.int64)
        np.cumsum(seg_tot_a[:, t], out=bnd[1:])

        def dequant_one(c):
            ta = int(seg_tot_a[c, t])
            seg = picked[bnd[c] : bnd[c] + ta]
            np.multiply(
                vals[c].reshape(cap_a, M)[:ta].astype(np.float32),
                float(vals[N_CORES + c].reshape(-1)[0]),
                out=seg,
            )
            seg += bc[None, :]

        list(pool.map(dequant_one, range(N_CORES)))
        res[va & (sample_chunk == t)[:, None]] = picked
        marks.append(time.perf_counter())
    _dbg(
        "chunks "
        + " ".join(f"{b - a:.3f}" for a, b in zip(marks, marks[1:]))
        + f" total:{marks[-1] - marks[0]:.3f}"
    )
    return res


def _forward_np(entities, entity_mask, W1, b1, Wqkv, Wout, bout, W2, b2):
    bs, ne, _ = entities.shape
    x1 = np.maximum(entities @ W1 + b1, 0.0)
    em = entity_mask.astype(np.float32)
    am = em[:, :N_AGENTS]
    attn_mask = 1.0 - np.einsum("bi,bj->bij", 1.0 - am, 1.0 - em)
    qkv = x1 @ Wqkv
    q, k, v = np.split(qkv, 3, axis=-1)
    q = q[:, :N_AGENTS]

    def heads(t):
        b, n, _ = t.shape
        return t.reshape(b, n, N_HEADS, HD).transpose(0, 2, 1, 3)

    qh, kh, vh = heads(q), heads(k), heads(v)
    logits = np.einsum("bhqd,bhkd->bhqk", qh, kh) / np.sqrt(np.float32(HD))
    logits = np.where(attn_mask[:, None] > 0, -np.inf, logits)
    m = np.max(logits, axis=-1, keepdims=True)
    m = np.where(np.isinf(m), 0.0, m)
    ex = np.exp(logits - m)
    s = np.sum(ex, axis=-1, keepdims=True)
    w = np.where(s > 0, ex / np.where(s == 0, 1.0, s), 0.0)
    attn = np.einsum("bhqk,bhkd->bhqd", w, vh)
    attn = attn.transpose(0, 2, 1, 3).reshape(bs, N_AGENTS, E)
    x2 = attn @ Wout + bout
    x2 = np.where(am[:, :, None] > 0, 0.0, x2)
    x3 = x2 @ W2 + b2
    x3 = np.where(am[:, :, None] > 0, 0.0, x3)
    return x3.astype(np.float32)


def kernel(entities, entity_mask, W1, b1, Wqkv, Wout, bout, W2, b2):
    args = (
        np.asarray(entities, np.float32),
        np.asarray(entity_mask, np.int32),
        np.asarray(W1, np.float32),
        np.asarray(b1, np.float32),
        np.asarray(Wqkv, np.float32),
        np.asarray(Wout, np.float32),
        np.asarray(bout, np.float32),
        np.asarray(W2, np.float32),
        np.asarray(b2, np.float32),
    )
    try:
        return _run_packed(*args)
    except Exception as e:
        _dbg(f"packed path failed: {type(e).__name__}: {e}")
        return _forward_np(*args)


# revision 25
# speedup vs baseline: 1.2976x; 1.1124x over previous
"""AttentionHyperNet kernel — data-parallel across 8 NeuronCores.

Wire-optimized path: the tunnel to the device pod is the bottleneck
(~80 MB/s up, ~50 MB/s down, ~70 ms per RPC round trip), so the kernel
  * drops masked entity rows on the host (they cannot affect the
    output: masked agents are zeroed, masked entities get -inf
    attention logits) and ships only the valid rows, flat-packed, as
    float16 — the device re-expands them to a padded per-sample layout
    with a gather,
  * folds Wout@W2 into one (128,32) matrix on the host (no
    nonlinearity between them) and moves the output bias to the host,
  * packs payload + params + offsets/indices into ONE float16 buffer
    per core (no replicated transfers, one put per core, built and
    uploaded from a thread pool), and runs the batch as two pipelined
    chunks per core so chunk B's upload overlaps chunk A's compute
    and chunk A's download,
  * computes in f32 on device (exp-masked softmax without the
    max-subtraction passes; logit scale folded into the q weights),
    quantizes the compacted valid output rows to int8 with a per-core
    scale (error <= 1/254 of the per-core max, well under the 2e-2
    global-max tolerance),
  * scatters the dequantized rows into the full fp32 (4096, 64, 32)
    output on the host.

Self-contained: no sibling imports, shapes hardcoded.
"""

import os
import sys
import time

import numpy as np

N_AGENTS = 64
N_HEADS = 4
N_CORES = 8
BS = 4096
NE = 128
FD = 19
E = 128
M = 32
SH = BS // N_CORES
HD = E // N_HEADS

# metadata layout (f16 elements), all sections 128-aligned
N_PAR = FD * E + E + E * 3 * E + E * M
N_PAR_PAD = ((N_PAR + 127) // 128) * 128
O_CNT = N_PAR_PAD  # then: cnt (sh), off_hi (sh), off_lo (sh), idx hi/lo
N_CHUNKS = 2

_DEBUG = bool(os.environ.get("BASSKERNEL_DEBUG"))


def _dbg(msg):
    if _DEBUG:
        print(f"[kernel] {msg}", file=sys.stderr, flush=True)


def _round_up(x, m):
    return ((int(x) + m - 1) // m) * m


_JAX_STATE = {}
_FWD_CACHE = {}


def _jax_state():
    if _JAX_STATE:
        return _JAX_STATE
    import jax
    from jax.sharding import Mesh, NamedSharding, PartitionSpec as P

    devs = jax.devices()[:N_CORES]
    if len(devs) < N_CORES:
        raise RuntimeError("need 8 cores")
    mesh = Mesh(np.array(devs), ("b",))
    _JAX_STATE["jax"] = jax
    _JAX_STATE["mesh"] = mesh
    _JAX_STATE["shard"] = NamedSharding(mesh, P("b"))
    _JAX_STATE["P"] = P
    return _JAX_STATE


def _get_fwd(sh, cap_e, cap_a, ne_pad, na_pad):
    key = (sh, cap_e, cap_a, ne_pad, na_pad)
    fn = _FWD_CACHE.get(key)
    if fn is not None:
        return fn
    st = _jax_state()
    jax = st["jax"]
    mesh = st["mesh"]
    P = st["P"]
    import jax.numpy as jnp

    from jax.experimental.shard_map import shard_map

    n_ent = cap_e * FD

    def core_fwd(buf):  # (1, n_ent + meta) f16, payload then metadata
        meta = buf.reshape(-1)
        ent = meta[:n_ent].reshape(cap_e, FD)
        pos = [n_ent]

        def take(n, shape):
            v = meta[pos[0] : pos[0] + n].reshape(shape)
            pos[0] += n
            return v

        W1 = take(FD * E, (FD, E)).astype(jnp.float32)
        b1 = take(E, (E,)).astype(jnp.float32)
        Wqkv = take(E * 3 * E, (E, 3 * E)).astype(jnp.float32)
        Wc = take(E * M, (E, M)).astype(jnp.float32)
        pos[0] = n_ent + O_CNT
        cnt = take(sh, (sh,)).astype(jnp.float32)
        off = (
            take(sh, (sh,)).astype(jnp.float32) * 256.0
            + take(sh, (sh,)).astype(jnp.float32)
        ).astype(jnp.int32)
        oidx = (
            take(cap_a, (cap_a,)).astype(jnp.float32) * 256.0
            + take(cap_a, (cap_a,)).astype(jnp.float32)
        ).astype(jnp.int32)

        gidx = jnp.clip(
            off[:, None] + jnp.arange(ne_pad, dtype=jnp.int32)[None, :],
            0,
            cap_e - 1,
        )
        pe = ent.astype(jnp.float32)[gidx]  # (sh, ne_pad, FD) f32
        x1 = jax.nn.relu(pe @ W1 + b1)
        qkv = x1 @ Wqkv  # (sh, ne_pad, 3E)
        q = qkv[:, :na_pad, :E]
        k = qkv[:, :, E : 2 * E]
        v = qkv[:, :, 2 * E :]
        qh = q.reshape(sh, na_pad, N_HEADS, HD)
        kh = k.reshape(sh, ne_pad, N_HEADS, HD)
        vh = v.reshape(sh, ne_pad, N_HEADS, HD)
        logits = jnp.einsum("sqhd,skhd->shqk", qh, kh)
        kmask = (
            jnp.arange(ne_pad, dtype=jnp.float32)[None, :] < cnt[:, None]
        ).astype(jnp.float32)
        ex = jnp.exp(logits) * kmask[:, None, None, :]
        w = ex / (jnp.sum(ex, axis=-1, keepdims=True) + 1e-30)
        attn = jnp.einsum("shqk,skhd->sqhd", w, vh).reshape(sh, na_pad, E)
        x3 = jnp.einsum("sqe,em->sqm", attn, Wc)
        flat = x3.reshape(sh * na_pad, M)
        out = flat[oidx]  # (cap_a, M) f32, pad slots duplicate row 0
        smax = jnp.max(jnp.abs(out))
        scale = jnp.maximum(smax, 1e-20) * (1.0 / 127.0)
        qv = jnp.clip(jnp.rint(out / scale), -127, 127).astype(jnp.int8)
        return qv[None], scale.reshape(1, 1)

    fwd = jax.jit(
        shard_map(
            core_fwd,
            mesh=mesh,
            in_specs=P("b"),
            out_specs=(P("b"), P("b")),
            check_rep=False,
        )
    )
    _FWD_CACHE[key] = fwd
    return fwd


def _run_packed(entities, entity_mask, W1, b1, Wqkv, Wout, bout, W2, b2):
    st = _jax_state()
    jax = st["jax"]
    shard = st["shard"]

    t0 = time.perf_counter()
    ent = np.ascontiguousarray(entities, np.float32).reshape(BS, NE, FD)
    valid = np.ascontiguousarray(entity_mask).reshape(BS, NE) == 0
    cnt_e = valid.sum(1).astype(np.int64)
    va = valid[:, :N_AGENTS]
    cnt_a = va.sum(1).astype(np.int64)
    shc = SH // N_CHUNKS  # samples per core per chunk
    ce = cnt_e.reshape(N_CORES, N_CHUNKS, shc)
    ca = cnt_a.reshape(N_CORES, N_CHUNKS, shc)
    seg_tot_e = ce.sum(2)  # (C, T)
    seg_tot_a = ca.sum(2)

    ne_pad = max(8, _round_up(ce.max(), 8))
    na_pad = max(8, _round_up(ca.max(), 8))
    cap_e = max(2048, _round_up(seg_tot_e.max(), 2048))
    cap_a = max(1024, _round_up(seg_tot_a.max(), 1024))
    n_ent = cap_e * FD
    o_off_hi = O_CNT + shc
    o_off_lo = o_off_hi + shc
    o_idx = o_off_lo + shc
    meta_total = _round_up(o_idx + 2 * cap_a + 256, 128)

    from concurrent.futures import ThreadPoolExecutor

    devs = st["mesh"].devices.reshape(-1)
    if "pool" not in st:
        st["pool"] = ThreadPoolExecutor(max_workers=N_CORES)
    pool = st["pool"]

    # host-folded params (shared by all chunks)
    Wc = (
        np.asarray(Wout, np.float64) @ np.asarray(W2, np.float64)
    ).astype(np.float32)
    bc = (
        np.asarray(bout, np.float64) @ np.asarray(W2, np.float64)
        + np.asarray(b2, np.float64)
    ).astype(np.float32)
    Wqkv_s = np.asarray(Wqkv, np.float32).copy()
    Wqkv_s[:, :E] *= 1.0 / np.sqrt(float(HD))  # fold logit scale into q
    params16 = np.concatenate(
        [
            np.asarray(W1, np.float32).ravel(),
            np.asarray(b1, np.float32).ravel(),
            Wqkv_s.ravel(),
            Wc.ravel(),
        ]
    ).astype(np.float16)

    fwd = _get_fwd(shc, cap_e, cap_a, ne_pad, na_pad)

    total = n_ent + meta_total
    bufcache = st.setdefault("bufcache", {})

    def pack_one(c, t):
        # reuse the pack buffer across calls; device_put copies
        # synchronously in this thread, so mutation next call is safe
        ckey = (c, t, total)
        cached = bufcache.get(ckey)
        if cached is None:
            arr, last_nfd, last_ta = np.zeros((1, total), np.float16), 0, 0
        else:
            arr, last_nfd, last_ta = cached
        g0 = c * SH + t * shc
        rows = ent[g0 : g0 + shc][valid[g0 : g0 + shc]]
        n = rows.shape[0]
        nfd = n * FD
        if last_nfd > nfd:
            arr[0, nfd:last_nfd] = 0
        if n:
            arr[0, :nfd] = rows.astype(np.float16).reshape(-1)
        mt = arr[0, n_ent:]
        mt[:N_PAR] = params16
        mt[O_CNT : O_CNT + shc] = ce[c, t].astype(np.float16)
        seg_cum = np.zeros(shc + 1, np.int64)
        np.cumsum(ce[c, t], out=seg_cum[1:])
        off = seg_cum[:-1]
        mt[o_off_hi : o_off_hi + shc] = (off >> 8).astype(np.float16)
        mt[o_off_lo : o_off_lo + shc] = (off & 255).astype(np.float16)
        ta = int(seg_tot_a[c, t])
        if last_ta > ta:
            mt[o_idx + ta : o_idx + last_ta] = 0
            mt[o_idx + cap_a + ta : o_idx + cap_a + last_ta] = 0
        if ta:
            i_ids = np.repeat(np.arange(shc), ca[c, t])
            cum_a = np.zeros(shc + 1, np.int64)
            np.cumsum(ca[c, t], out=cum_a[1:])
            j_ids = np.arange(ta) - np.repeat(cum_a[:-1], ca[c, t])
            idx = i_ids * na_pad + j_ids
            mt[o_idx : o_idx + ta] = (idx >> 8).astype(np.float16)
            mt[o_idx + cap_a : o_idx + cap_a + ta] = (idx & 255).astype(
                np.float16
            )
        bufcache[ckey] = (arr, nfd, ta)
        return jax.device_put(arr, devs[c])

    chunk_outs = []
    marks = [t0]
    for t in range(N_CHUNKS):
        futs = [pool.submit(pack_one, c, t) for c in range(N_CORES)]
        singles = [f.result() for f in futs]
        g_buf = jax.make_array_from_single_device_arrays(
            (N_CORES, total), shard, singles
        )
        outq, outs = fwd(g_buf)
        chunk_outs.append((outq, outs))
        marks.append(time.perf_counter())

    # issue all host copies, then drain chunk by chunk
    all_datas = []
    for outq, outs in chunk_outs:
        qs = sorted(
            outq.addressable_shards, key=lambda s: s.index[0].start or 0
        )
        ss = sorted(
            outs.addressable_shards, key=lambda s: s.index[0].start or 0
        )
        ds = [s.data for s in qs] + [s.data for s in ss]
        for d in ds:
            d.copy_to_host_async()
        all_datas.append(ds)
    res = np.zeros((BS, N_AGENTS, M), np.float32)
    sample_chunk = (np.arange(BS) % SH) // shc  # chunk id per sample
    marks.append(time.perf_counter())
    for t in range(N_CHUNKS):
        vals = [np.asarray(d) for d in all_datas[t]]
        tot_a = int(seg_tot_a[:, t].sum())
        picked = np.empty((tot_a, M), np.float32)
        bnd = np.zeros(N_CORES + 1, np.int64)
        np.cumsum(seg_tot_a[:, t], out=bnd[1:])

        def dequant_one(c):
            ta = int(seg_tot_a[c, t])
            seg = picked[bnd[c] : bnd[c] + ta]
            np.multiply(
                vals[c].reshape(cap_a, M)[:ta].astype(np.float32),
                float(vals[N_CORES + c].reshape(-1)[0]),
                out=seg,
            )
            seg += bc[None, :]

        list(pool.map(dequant_one, range(N_CORES)))
        res[va & (sample_chunk == t)[:, None]] = picked
        marks.append(time.perf_counter())
    _dbg(
        "chunks "
        + " ".join(f"{b - a:.3f}" for a, b in zip(marks, marks[1:]))
        + f" total:{marks[-1] - marks[0]:.3f}"
    )
    return res


def _forward_np(entities, entity_mask, W1, b1, Wqkv, Wout, bout, W2, b2):
    bs, ne, _ = entities.shape
    x1 = np.maximum(entities @ W1 + b1, 0.0)
    em = entity_mask.astype(np.float32)
    am = em[:, :N_AGENTS]
    attn_mask = 1.0 - np.einsum("bi,bj->bij", 1.0 - am, 1.0 - em)
    qkv = x1 @ Wqkv
    q, k, v = np.split(qkv, 3, axis=-1)
    q = q[:, :N_AGENTS]

    def heads(t):
        b, n, _ = t.shape
        return t.reshape(b, n, N_HEADS, HD).transpose(0, 2, 1, 3)

    qh, kh, vh = heads(q), heads(k), heads(v)
    logits = np.einsum("bhqd,bhkd->bhqk", qh, kh) / np.sqrt(np.float32(HD))
    logits = np.where(attn_mask[:, None] > 0, -np.inf, logits)
    m = np.max(logits, axis=-1, keepdims=True)
    m = np.where(np.isinf(m), 0.0, m)
    ex = np.exp(logits - m)
    s = np.sum(ex, axis=-1, keepdims=True)
    w = np.where(s > 0, ex / np.where(s == 0, 1.0, s), 0.0)
    attn = np.einsum("bhqk,bhkd->bhqd", w, vh)
    attn = attn.transpose(0, 2, 1, 3).reshape(bs, N_AGENTS, E)
    x2 = attn @ Wout + bout
    x2 = np.where(am[:, :, None] > 0, 0.0, x2)
    x3 = x2 @ W2 + b2
    x3 = np.where(am[:, :, None] > 0, 0.0, x3)
    return x3.astype(np.float32)


def kernel(entities, entity_mask, W1, b1, Wqkv, Wout, bout, W2, b2):
    args = (
        np.asarray(entities, np.float32),
        np.asarray(entity_mask, np.int32),
        np.asarray(W1, np.float32),
        np.asarray(b1, np.float32),
        np.asarray(Wqkv, np.float32),
        np.asarray(Wout, np.float32),
        np.asarray(bout, np.float32),
        np.asarray(W2, np.float32),
        np.asarray(b2, np.float32),
    )
    try:
        return _run_packed(*args)
    except Exception as e:
        _dbg(f"packed path failed: {type(e).__name__}: {e}")
        return _forward_np(*args)
